# revision 67
# baseline (speedup 1.0000x reference)
"""GQA attention kernel for 8 Trainium2 NeuronCores (v3).

Sharding: core c handles batch b = c//4, query rows [512*(c%4), 512*(c%4)+512).
Each core computes K/V for its batch's full (rolled) sequence, all 16 heads of
attention for its 512 query rows, and the final projection. No collectives.

v3 is a scheduling rewrite of v2 driven by TimelineSim engine occupancy:
the Act engine's exp stream (16 units x 8 x [128,1024] ~ 133us) is the hard
floor, and the PE's ~150us of matmuls must hide almost entirely under it.

  - QKV/norm/proj work is emitted as <=1us "fill granules" interleaved into
    the score units' w-slots so the PE tracks the exp stream; the first two
    head-tiles are emitted in w-halves so the Act stream backfills across
    units while the xk DMA chunks and k-norm chains land;
  - PSUM: ps 2x[128,1024] + aux 1x[128,512] (qkv/proj) + nrm 1x[128,512]
    (t2p/prb) + po 1x[128,4,65] + pkb 1x[66,512] = 16KB exactly;
  - rsqrt chains batched: up to 3 subs' sum-of-squares accumulate into one
    [66,512] PSUM tile (rows 32i, zero-padded mask stationaries) so one
    5-op DVE Newton chain serves a whole batch; prologue batches use Act
    Sqrt (before the Exp table load) instead;
  - prologue raw/sq ride the idle Act engine (Identity/Square with bias);
  - v bias folded into the output bias (softmax rows sum to 1 =>
    attn@(v+b) = attn@v + b), proj bias folded into a broadcast SBUF tile
    added during the proj_a PSUM->SBUF copy: zero bias matmuls;
  - startup DMAs chunked and ordered along the critical path; small f32
    constants packed into one [128,464] f32r transfer;
  - projection: mt0-5 accumulate as fills in units 6-7 (+ bias via bpb);
    the tail (mt6,7 + foA via identity matmul) fans out across all free
    PSUM rings, PSUM->SBUF copies alternating Act (Copy) and DVE;
  - bf16 output DMA (halves the final writeback).
"""

import numpy as np

import concourse.bass as bass
import concourse.tile as tile
from concourse import bacc, mybir
from concourse import bass_utils

B, N, E = 2, 2048, 1024
H, KV, D = 16, 4, 64
R = 512            # query rows per core
EPS = 1e-6
F32 = mybir.dt.float32
F32R = mybir.dt.float32r
U32 = mybir.dt.uint32
BF16 = mybir.dt.bfloat16
AF = mybir.ActivationFunctionType
ALU = mybir.AluOpType

# head order: tile t holds (HEAD_ORDER[2t] at rows 0:64, HEAD_ORDER[2t+1] at 64:128)
HEAD_ORDER = [0, 4, 1, 5, 2, 6, 3, 7, 8, 12, 9, 13, 10, 14, 11, 15]


def _emit(tc, dr):
    nc = tc.nc
    with (
        tc.tile_pool(name="pers", bufs=1) as pers,
        tc.tile_pool(name="work", bufs=2) as wk,
        tc.tile_pool(name="wqs", bufs=2) as wqs,
        tc.tile_pool(name="ets", bufs=21) as ets,
        tc.tile_pool(name="outs", bufs=2) as outs,
        tc.tile_pool(name="psp", bufs=2, space=bass.MemorySpace.PSUM) as psp,
        tc.tile_pool(name="auxp", bufs=1, space=bass.MemorySpace.PSUM) as auxp,
        tc.tile_pool(name="nrmp", bufs=1, space=bass.MemorySpace.PSUM) as nrmp,
        tc.tile_pool(name="pop", bufs=1, space=bass.MemorySpace.PSUM) as pop,
        tc.tile_pool(name="pkbp", bufs=1, space=bass.MemorySpace.PSUM) as pkbp,
    ):
        # ---------------- persistent tiles ----------------
        kt_til = [pers.tile([128, N], BF16, tag=f"kt{i}", name=f"ktt{i}")
                  for i in range(2)]
        qt_til = [pers.tile([128, 2, R], BF16, tag=f"qt{i}", name=f"qtt{i}")
                  for i in range(4)]
        vt_t = pers.tile([128, 16, 4, 65], BF16, tag="vt")  # v + ones col per g
        ot_t = pers.tile([128, 8, R], BF16, tag="ot")      # attn out (m, q)
        pj_t = pers.tile([128, 8, 1024], BF16, tag="pj")   # proj weights
        spk_t = pers.tile([128, 464], F32R, tag="spk")     # packed constants
        bk_t = spk_t[:, 0:2].bitcast(F32)
        bq_t = spk_t[:, 2:10].bitcast(F32)
        smk_t = spk_t[:, 10:208].rearrange("p (v c) -> p v c", v=3)
        p2_t = spk_t[:, 208:336]
        bcm_t = spk_t[:, 336:464]
        bpb_t = pers.tile([128, 2, 512], BF16, tag="bpb")  # proj+v bias bcast
        eps_t = pers.tile([128, 1], F32, tag="eps")
        kmag_t = pers.tile([128, 512], U32, tag="kmag")    # 0x5f3759df
        id_t = pers.tile([128, 128], BF16, tag="id")       # identity (tail)

        xk_t = pers.tile([128, 8, N], BF16, tag="xk")
        wk_t = pers.tile([128, 8, 256], BF16, tag="wk")
        wv_t = pers.tile([128, 8, 256], BF16, tag="wv")
        ck_t = pers.tile([128, N], BF16, tag="ck")    # cos*w for K cols
        skp_t = pers.tile([128, N], BF16, tag="skp")  # permuted sign*sin*w for K
        cq_t = pers.tile([128, R], BF16, tag="cq")
        sqp_t = pers.tile([128, R], BF16, tag="sqp")

        nc.vector.memset(vt_t[:, :, :, 64:65], 1.0)
        nc.vector.memset(kmag_t, 0x5F3759DF)
        nc.vector.memset(eps_t, 64.0 * EPS)

        # ---------- startup DMAs, ordered along the critical path ----------
        xr = dr["xfT"].rearrange("(e p) n -> p e n", p=128)
        nc.sync.dma_start(out=wk_t, in_=dr["wkT"].rearrange("(e p) m -> p e m", p=128))
        nc.sync.dma_start(out=xk_t[:, :, 0:512], in_=xr[:, :, 0:512])
        wq_c0 = wqs.tile([128, 8, 256], BF16, tag="wqc", name="wqc0")
        wqr = dr["wqT"].rearrange("(e p) m -> p e m", p=128)
        nc.sync.dma_start(out=wq_c0, in_=wqr[:, :, 0:256])
        nc.sync.dma_start(out=skp_t[:, 0:1024], in_=dr["skpT"][:, 0:1024])
        nc.sync.dma_start(out=ck_t[:, 0:1024], in_=dr["ckT"][:, 0:1024])
        nc.sync.dma_start(out=spk_t, in_=dr["spk"])
        nc.sync.dma_start(out=cq_t, in_=dr["cqT"])
        nc.sync.dma_start(out=sqp_t, in_=dr["sqpT"])
        nc.sync.dma_start(out=xk_t[:, :, 512:1024], in_=xr[:, :, 512:1024])
        nc.sync.dma_start(out=xk_t[:, :, 1024:1536], in_=xr[:, :, 1024:1536])
        nc.sync.dma_start(out=xk_t[:, :, 1536:2048], in_=xr[:, :, 1536:2048])
        nc.sync.dma_start(out=wv_t, in_=dr["wvT"].rearrange("(e p) m -> p e m", p=128))
        nc.sync.dma_start(out=ck_t[:, 1024:2048], in_=dr["ckT"][:, 1024:2048])
        nc.sync.dma_start(out=skp_t[:, 1024:2048], in_=dr["skpT"][:, 1024:2048])
        nc.sync.dma_start(out=bpb_t, in_=dr["bpb"])
        nc.sync.dma_start(out=id_t, in_=dr["id128"])

        # ---------------- norm machinery ----------------
        # Per 128-row m-block x 512-token "sub": raw = psum+bias on DVE;
        # squares (gpsimd fast / Act Square slow); sum-of-squares via a
        # zero-padded mask matmul accumulating batch row-pair 2i; rope
        # products u (gpsimd) and t1 (gpsimd); rsqrt for a whole batch in
        # one 5-op DVE Newton chain ([8,512]: free size stays 512); then
        # t2p = P@u, s = t2p+t1, prb = bcast(rsv), out = s*prb.
        class Batch:
            def __init__(self, nm, n_subs, slow):
                self.nm = nm
                self.n = n_subs
                self.slow = slow
                self.next_i = 0
                self.tile = None
                self.rsv = None

        st_u = {}
        st_t1 = {}
        st_s = {}

        def norm_front(key, pr, bias_ap, cs_ap, sp_ap, batch, act=False):
            raw = wk.tile([128, 512], F32, tag="raw", bufs=2, name=f"raw{key}")
            sq = wk.tile([128, 512], F32R, tag="sq", bufs=2, name=f"sq{key}")
            if act:
                # prologue subs: psum+bias reads on the (idle) Act engine
                nc.scalar.activation(out=raw, in_=pr, func=AF.Identity,
                                     bias=bias_ap, scale=1.0)
                nc.scalar.activation(out=sq, in_=pr, func=AF.Square,
                                     bias=bias_ap)
            else:
                nc.vector.tensor_scalar_add(out=raw, in0=pr, scalar1=bias_ap)
                nc.vector.tensor_mul(sq, raw, raw)
            i = batch.next_i
            batch.next_i += 1
            if i == 0:
                batch.tile = pkbp.tile([66, 512], F32, tag="pkb",
                                       name=f"pkb{batch.nm}")
            nc.tensor.matmul(batch.tile, smk_t[:, i, :], sq,
                             start=(i == 0), stop=(i == batch.n - 1))
            u = wk.tile([128, 512], F32R, tag="u", bufs=3, name=f"u{key}")
            nc.gpsimd.tensor_mul(u, raw, sp_ap)
            t1 = wk.tile([128, 512], F32, tag="t1", bufs=3, name=f"t1{key}")
            nc.gpsimd.tensor_mul(t1, raw, cs_ap)
            st_u[key] = u
            st_t1[key] = t1
            return i

        def chain(batch):
            pk = batch.tile
            r = 32 * (batch.n - 1) + 2
            rv = wk.tile([66, 512], F32R, tag="rv", bufs=2,
                         name=f"rv{batch.nm}")
            if batch.slow:
                sdk = wk.tile([66, 512], F32, tag="sdk", bufs=1,
                              name=f"sdk{batch.nm}")
                nc.scalar.activation(out=sdk[0:r], in_=pk[0:r], func=AF.Sqrt,
                                     bias=eps_t[0:r], scale=1.0)
                with nc.allow_low_precision(reason="bf16-level norm"):
                    nc.vector.reciprocal(out=rv[0:r], in_=sdk[0:r])
            else:
                sh = wk.tile([66, 512], U32, tag="sh", bufs=1,
                             name=f"sh{batch.nm}")
                nc.vector.tensor_scalar(out=sh[0:r], in0=pk[0:r].bitcast(U32),
                                        scalar1=1, scalar2=None,
                                        op0=ALU.logical_shift_right)
                y0 = wk.tile([66, 512], U32, tag="y0", bufs=1,
                             name=f"y0{batch.nm}")
                nc.vector.tensor_tensor(out=y0[0:r], in0=kmag_t[0:r],
                                        in1=sh[0:r], op=ALU.subtract)
                y2 = wk.tile([66, 512], F32, tag="y2", bufs=1,
                             name=f"y2{batch.nm}")
                nc.vector.tensor_mul(y2[0:r], y0[0:r].bitcast(F32),
                                     y0[0:r].bitcast(F32))
                nb = wk.tile([66, 512], F32, tag="nb", bufs=1,
                             name=f"nb{batch.nm}")
                nc.vector.scalar_tensor_tensor(out=nb[0:r], in0=pk[0:r],
                                               scalar=-0.5, in1=y2[0:r],
                                               op0=ALU.mult, op1=ALU.mult)
                nc.vector.scalar_tensor_tensor(out=rv[0:r], in0=nb[0:r],
                                               scalar=1.5,
                                               in1=y0[0:r].bitcast(F32),
                                               op0=ALU.add, op1=ALU.mult)
            batch.rsv = rv

        def norm_f2(key):
            u = st_u.pop(key)
            t2p = nrmp.tile([128, 512], F32, tag="nrm", name=f"t2p{key}")
            nc.tensor.matmul(t2p, p2_t, u, start=True, stop=True)
            s = wk.tile([128, 512], F32, tag="s", bufs=2, name=f"s{key}")
            nc.vector.scalar_tensor_tensor(out=s, in0=t2p, scalar=0.0,
                                           in1=st_t1.pop(key),
                                           op0=ALU.add, op1=ALU.add)
            st_s[key] = s

        def norm_f3(key, batch, i, out_ap):
            prb = nrmp.tile([128, 512], F32, tag="nrm", name=f"prb{key}")
            nc.tensor.matmul(prb, bcm_t[32 * i:32 * i + 2, :],
                             batch.rsv[32 * i:32 * i + 2],
                             start=True, stop=True)
            nc.vector.tensor_mul(out_ap, st_s.pop(key), prb)

        # ---------------- k / q / v sub emitters ----------------
        aux_store = {}
        sub_meta = {}

        def k_mm(kt, nb, part):
            key = f"k{kt}{nb}"
            if part == 0:
                aux_store[key] = auxp.tile([128, 512], F32, tag="aux",
                                           name=f"pk{key}")
            pr = aux_store[key]
            for e in range(4 * part, 4 * part + 4):
                nc.tensor.matmul(pr, wk_t[:, e, 128 * kt:128 * (kt + 1)],
                                 xk_t[:, e, 512 * nb:512 * (nb + 1)],
                                 start=(e == 0), stop=(e == 7))

        def k_front(kt, nb, batch, act=False):
            key = f"k{kt}{nb}"
            i = norm_front(key, aux_store.pop(key), bk_t[:, kt:kt + 1],
                           ck_t[:, 512 * nb:512 * (nb + 1)],
                           skp_t[:, 512 * nb:512 * (nb + 1)], batch, act)
            sub_meta[key] = (batch, i)

        def k_back2(kt, nb):
            norm_f2(f"k{kt}{nb}")

        def k_back3(kt, nb):
            key = f"k{kt}{nb}"
            batch, i = sub_meta.pop(key)
            norm_f3(key, batch, i,
                    kt_til[kt][:, 512 * nb:512 * (nb + 1)])

        wq_store = {0: wq_c0}

        def wq_dma(qp):
            wq_c = wqs.tile([128, 8, 256], BF16, tag="wqc", name=f"wqc{qp}")
            nc.sync.dma_start(out=wq_c, in_=wqr[:, :, 256 * qp:256 * (qp + 1)])
            wq_store[qp] = wq_c

        def q_mm(qp, j, part, pool=None):
            key = f"q{qp}{j}"
            if part == 0:
                aux_store[key] = (pool or auxp).tile(
                    [128, 512], F32, tag="po" if pool else "aux",
                    name=f"pq{key}")
            pr = aux_store[key]
            wq_c = wq_store[qp]
            for e in range(4 * part, 4 * part + 4):
                nc.tensor.matmul(pr, wq_c[:, e, 128 * j:128 * (j + 1)],
                                 xk_t[:, e, 0:R],
                                 start=(e == 0), stop=(e == 7))

        def q_front(qp, j, batch, act=False):
            key = f"q{qp}{j}"
            i = norm_front(key, aux_store.pop(key),
                           bq_t[:, 2 * qp + j:2 * qp + j + 1],
                           cq_t, sqp_t, batch, act)
            sub_meta[key] = (batch, i)

        def q_back2(qp, j):
            norm_f2(f"q{qp}{j}")

        def q_back3(qp, j):
            key = f"q{qp}{j}"
            batch, i = sub_meta.pop(key)
            norm_f3(key, batch, i, qt_til[qp][:, j, :])

        def v_mm(gp, q4, cpair):
            key = f"v{gp}{q4}"
            if cpair == 0:
                aux_store[key] = auxp.tile([128, 512], F32, tag="aux",
                                           name=f"pv{key}")
            pv = aux_store[key]
            for c in range(2 * cpair, 2 * cpair + 2):
                nch = 4 * q4 + c
                for e in range(8):
                    nc.tensor.matmul(pv[:, 128 * c:128 * (c + 1)],
                                     xk_t[:, e, 128 * nch:128 * (nch + 1)],
                                     wv_t[:, e, 128 * gp:128 * (gp + 1)],
                                     start=(e == 0), stop=(e == 7))
            if cpair == 1:
                pv = aux_store.pop(key)
                nc.vector.tensor_copy(
                    out=vt_t[:, 4 * q4:4 * q4 + 4, 2 * gp:2 * gp + 2, 0:64],
                    in_=pv.rearrange("p (c g x) -> p c g x", c=4, g=2))

        # ---------------- stage-2 unit emitters ----------------
        et_store = {}
        od_store = {}

        def score_unit(t, r01, fills=(), ws=tuple(range(8))):
            ktile = t // 4
            h = HEAD_ORDER[2 * t + r01]
            gq = h // 4
            prow = 64 * (gq % 2)
            assert gq // 2 == ktile and prow == 64 * r01
            qn_h = qt_til[t // 2][prow:prow + 64, t % 2, :]
            etl = et_store.setdefault((t, r01), [None] * 8)
            fi = 0
            for w in ws:
                ps = psp.tile([128, 1024], F32, tag="ps", name=f"ps{t}{r01}{w}")
                for c in range(2):
                    nch = 2 * w + c
                    nc.tensor.matmul(
                        ps[:, 512 * c:512 * (c + 1)],
                        kt_til[ktile][prow:prow + 64, 128 * nch:128 * (nch + 1)],
                        qn_h, start=True, stop=True)
                et = ets.tile([128, 1024], BF16, tag="et", bufs=21,
                              name=f"et{t}{r01}{w}")
                etl[w] = et
                nc.scalar.activation(out=et, in_=ps, func=AF.Exp, scale=0.125)
                if fi < len(fills):
                    fills[fi]()
                    fi += 1
            for f in fills[fi:]:
                f()

        def attnv_unit(t, r01):
            h = HEAD_ORDER[2 * t + r01]
            gq = h // 4
            etl = et_store.pop((t, r01))
            if r01 == 0:
                od_store[t] = outs.tile([128, 4, 128], BF16, tag="od",
                                        name=f"od{t}")
            od = od_store[t]
            po = pop.tile([128, 4, 65], F32, tag="po", name=f"po{t}{r01}")
            for qc in range(4):
                for nch in range(16):
                    nc.tensor.matmul(
                        po[:, qc, :],
                        etl[nch // 2][:, 512 * (nch % 2) + 128 * qc:
                                      512 * (nch % 2) + 128 * (qc + 1)],
                        vt_t[:, nch, gq, :],
                        start=(nch == 0), stop=(nch == 15))
            rcp = outs.tile([128, 4, 1], F32, tag="rcp", name=f"rcp{t}{r01}")
            for qc in range(4):
                nc.vector.reciprocal(out=rcp[:, qc, :],
                                     in_=po[:, qc, 64:65])
                nc.vector.tensor_scalar_mul(
                    out=od[:, qc, 64 * r01:64 * r01 + 64],
                    in0=po[:, qc, 0:64], scalar1=rcp[:, qc, :])

        def transp_unit(t):
            od = od_store.pop(t)
            for qc in range(4):
                nc.sync.dma_start(out=ot_t[:, t, 128 * qc:128 * (qc + 1)],
                                  in_=od[:, qc, :], transpose=True)

        # ---------------- projection ----------------
        pjr = dr["pjT"].rearrange("(m p) e -> p m e", p=128)

        def pj_dma(mp):
            nc.sync.dma_start(out=pj_t[:, 2 * mp:2 * mp + 2, :],
                              in_=pjr[:, 2 * mp:2 * mp + 2, :])

        pf_store = {}
        foA = {}

        def pa_mm(half, rc, g):
            # accumulate mt 3g..3g+2 of proj for output block (half, rc)
            if g == 0:
                pf_store[(half, rc)] = auxp.tile([128, 512], F32, tag="aux",
                                                 name=f"pf{half}{rc}")
            pf = pf_store[(half, rc)]
            for mt in range(3 * g, 3 * g + 3):
                nc.tensor.matmul(pf, ot_t[:, mt, 128 * rc:128 * (rc + 1)],
                                 pj_t[:, mt, 512 * half:512 * (half + 1)],
                                 start=(mt == 0), stop=(mt == 5))
            if g == 1:
                pf = pf_store.pop((half, rc))
                fa = outs.tile([128, 512], BF16, tag="foa", bufs=8,
                               name=f"foa{half}{rc}")
                nc.vector.tensor_tensor(out=fa, in0=pf,
                                        in1=bpb_t[:, half, :], op=ALU.add)
                foA[(half, rc)] = fa

        pt_store = {}

        def pt_start(half, rc, pool=None, ptag=None):
            pf2 = (pool or psp).tile([128, 512], F32, tag=ptag or "ps",
                                     name=f"pt{half}{rc}")
            pt_store[(half, rc)] = pf2
            nc.tensor.matmul(pf2, ot_t[:, 6, 128 * rc:128 * (rc + 1)],
                             pj_t[:, 6, 512 * half:512 * (half + 1)],
                             start=True, stop=False)

        def pt_unit(half, rc, pool=None, ptag=None, on_act=None):
            if on_act is None:
                on_act = (half == 0)
            if (half, rc) not in pt_store:
                pt_start(half, rc, pool=pool, ptag=ptag)
            pf2 = pt_store.pop((half, rc))
            nc.tensor.matmul(pf2, ot_t[:, 7, 128 * rc:128 * (rc + 1)],
                             pj_t[:, 7, 512 * half:512 * (half + 1)],
                             start=False, stop=(not on_act))
            fo = outs.tile([128, 512], BF16, tag="fo", bufs=4,
                           name=f"fo{half}{rc}")
            if on_act:
                nc.tensor.matmul(pf2, id_t, foA.pop((half, rc)),
                                 start=False, stop=True)
                nc.scalar.activation(out=fo, in_=pf2, func=AF.Copy, scale=1.0)
            else:
                nc.vector.scalar_tensor_tensor(
                    out=fo, in0=pf2, scalar=0.0, in1=foA.pop((half, rc)),
                    op0=ALU.add, op1=ALU.add)
            nc.sync.dma_start(
                out=dr["out"][128 * rc:128 * (rc + 1),
                              512 * half:512 * (half + 1)],
                in_=fo)

        # ================= schedule =================
        bS = Batch("S", 2, slow=True)      # k00, q00 (Act sqrt prologue)
        bFa = Batch("Fa", 1, slow=False)   # k01
        bFb = Batch("Fb", 2, slow=False)   # q01, k02
        bFc = Batch("Fc", 1, slow=False)   # k03
        bK1a = Batch("K1a", 2, slow=False)  # k10, k11
        bK1b = Batch("K1b", 2, slow=False)  # k12, k13
        bQ1 = Batch("Q1", 2, slow=False)   # q10, q11
        bQ2 = Batch("Q2", 2, slow=False)   # q20, q21
        bQ3 = Batch("Q3", 2, slow=False)   # q30, q31

        # ---- prologue: minimal path to the first exp ----
        # bS (k00+q00, Act sqrt) gates the first exp; k01 (chain bFa) gates
        # w=2; q01/k02/k03 are xk2/3-DMA-gated and chain later (bF).
        k_mm(0, 0, 0); k_mm(0, 0, 1)
        q_mm(0, 0, 0, pool=pop); q_mm(0, 0, 1)
        k_front(0, 0, bS, act=True); q_front(0, 0, bS, act=True)
        chain(bS)
        k_back2(0, 0); k_back3(0, 0)
        q_back2(0, 0); q_back3(0, 0)
        k_mm(0, 1, 0); k_mm(0, 1, 1); k_front(0, 1, bFa, act=True)
        chain(bFa)
        k_back2(0, 1); k_back3(0, 1)
        # first score exps for keys 0:1024 — emitted before the k02/k03
        # blocks so the Act stream backfills while xk2/3 land and chain.
        score_unit(0, 0, ws=(0, 1, 2, 3))
        q_mm(0, 1, 0); q_mm(0, 1, 1); q_front(0, 1, bFb, act=True)
        score_unit(0, 1, ws=(0, 1, 2, 3), fills=(
            lambda: v_mm(0, 0, 0),
            lambda: v_mm(0, 0, 1),
        ))
        k_mm(0, 2, 0); k_mm(0, 2, 1); k_front(0, 2, bFb, act=True)
        chain(bFb)
        q_back2(0, 1); q_back3(0, 1)
        k_back2(0, 2); k_back3(0, 2)
        score_unit(1, 0, ws=(0, 1, 2, 3), fills=(
            lambda: wq_dma(1),
            lambda: v_mm(0, 1, 0),
            lambda: v_mm(0, 1, 1),
        ))
        k_mm(0, 3, 0); k_mm(0, 3, 1); k_front(0, 3, bFc, act=True)
        chain(bFc)
        k_back2(0, 3); k_back3(0, 3)
        score_unit(1, 1, ws=(0, 1, 2, 3), fills=(
            lambda: v_mm(0, 2, 0),
            lambda: v_mm(0, 2, 1),
            lambda: v_mm(0, 3, 0),
            lambda: v_mm(0, 3, 1),
        ))
        score_unit(0, 0, ws=(4, 5, 6, 7), fills=(
            lambda: q_mm(1, 0, 0),
            lambda: (q_mm(1, 0, 1), q_front(1, 0, bQ1)),
        ))
        score_unit(0, 1, ws=(4, 5, 6, 7), fills=(
            lambda: q_mm(1, 1, 0),
            lambda: (q_mm(1, 1, 1), q_front(1, 1, bQ1)),
        ))
        attnv_unit(0, 0)
        score_unit(1, 0, ws=(4, 5, 6, 7), fills=(
            lambda: chain(bQ1),
            lambda: q_back2(1, 0),
            lambda: q_back3(1, 0),
            lambda: q_back2(1, 1),
            lambda: q_back3(1, 1),
        ))
        attnv_unit(0, 1)
        transp_unit(0)
        score_unit(1, 1, ws=(4, 5, 6, 7))
        attnv_unit(1, 0)
        score_unit(2, 0, (
            lambda: wq_dma(2),
            lambda: k_mm(1, 0, 0),
            lambda: (k_mm(1, 0, 1), k_front(1, 0, bK1a)),
            lambda: k_mm(1, 1, 0),
            lambda: (k_mm(1, 1, 1), k_front(1, 1, bK1a)),
            lambda: chain(bK1a),
            lambda: k_back2(1, 0),
            lambda: k_back3(1, 0),
        ))
        attnv_unit(1, 1)
        transp_unit(1)
        score_unit(2, 1, (
            lambda: k_back2(1, 1),
            lambda: k_back3(1, 1),
            lambda: k_mm(1, 2, 0),
            lambda: (k_mm(1, 2, 1), k_front(1, 2, bK1b)),
            lambda: k_mm(1, 3, 0),
            lambda: (k_mm(1, 3, 1), k_front(1, 3, bK1b)),
            lambda: chain(bK1b),
            lambda: k_back2(1, 2),
        ))
        attnv_unit(2, 0)
        score_unit(3, 0, (
            lambda: k_back3(1, 2),
            lambda: k_back2(1, 3),
            lambda: k_back3(1, 3),
            lambda: q_mm(2, 0, 0),
            lambda: (q_mm(2, 0, 1), q_front(2, 0, bQ2)),
            lambda: q_mm(2, 1, 0),
            lambda: (q_mm(2, 1, 1), q_front(2, 1, bQ2)),
            lambda: chain(bQ2),
        ))
        attnv_unit(2, 1)
        transp_unit(2)
        score_unit(3, 1, (
            lambda: q_back2(2, 0),
            lambda: q_back3(2, 0),
            lambda: q_back2(2, 1),
            lambda: q_back3(2, 1),
            lambda: v_mm(1, 0, 0),
            lambda: v_mm(1, 0, 1),
            lambda: v_mm(1, 1, 0),
            lambda: v_mm(1, 1, 1),
        ))
        attnv_unit(3, 0)
        score_unit(4, 0, (
            lambda: v_mm(1, 2, 0),
            lambda: v_mm(1, 2, 1),
            lambda: v_mm(1, 3, 0),
            lambda: v_mm(1, 3, 1),
            lambda: wq_dma(3),
        ))
        attnv_unit(3, 1)
        transp_unit(3)
        score_unit(4, 1, (
            lambda: q_mm(3, 0, 0),
            lambda: (q_mm(3, 0, 1), q_front(3, 0, bQ3)),
            lambda: q_mm(3, 1, 0),
            lambda: (q_mm(3, 1, 1), q_front(3, 1, bQ3)),
        ))
        attnv_unit(4, 0)
        score_unit(5, 0, (
            lambda: chain(bQ3),
            lambda: q_back2(3, 0),
            lambda: q_back3(3, 0),
            lambda: q_back2(3, 1),
        ))
        attnv_unit(4, 1)
        transp_unit(4)
        score_unit(5, 1, (
            lambda: q_back3(3, 1),
            lambda: pj_dma(0),
            lambda: pj_dma(1),
        ))
        attnv_unit(5, 0)
        score_unit(6, 0, (
            lambda: pj_dma(2),
            lambda: pj_dma(3),
        ))
        attnv_unit(5, 1)
        transp_unit(5)
        score_unit(6, 1, (
            lambda: pa_mm(0, 0, 0),
            lambda: pa_mm(0, 0, 1),
            lambda: pa_mm(0, 1, 0),
            lambda: pa_mm(0, 1, 1),
            lambda: pa_mm(0, 2, 0),
            lambda: pa_mm(0, 2, 1),
            lambda: pa_mm(0, 3, 0),
            lambda: pa_mm(0, 3, 1),
        ))
        attnv_unit(6, 0)
        score_unit(7, 0, (
            lambda: pa_mm(1, 0, 0),
            lambda: pa_mm(1, 0, 1),
            lambda: pa_mm(1, 1, 0),
            lambda: pa_mm(1, 1, 1),
        ))
        attnv_unit(6, 1)
        transp_unit(6)
        score_unit(7, 1, (
            lambda: pa_mm(1, 2, 0),
            lambda: pa_mm(1, 2, 1),
            lambda: pa_mm(1, 3, 0),
            lambda: pa_mm(1, 3, 1),
        ))
        attnv_unit(7, 0)
        # pre-start tail chunks on already-free PSUM rings (mt6 accumulate)
        pt_start(0, 1, pool=nrmp, ptag="nrm")
        pt_start(0, 2, pool=auxp, ptag="aux")
        pt_start(0, 3, pool=pkbp, ptag="pkb")
        attnv_unit(7, 1)
        transp_unit(7)

        # ================= stage 3: projection tail =================
        pt_unit(0, 0)
        pt_unit(0, 1)
        pt_unit(0, 2)
        pt_unit(0, 3)
        pt_unit(1, 0, on_act=True)
        pt_unit(1, 1, pool=pop, ptag="po", on_act=True)
        pt_unit(1, 2, pool=auxp, ptag="aux", on_act=True)
        pt_unit(1, 3, pool=nrmp, ptag="nrm", on_act=True)


_CACHE = {}


def _get_nc():
    if "nc" in _CACHE:
        return _CACHE["nc"]
    nc = bacc.Bacc("TRN2", target_bir_lowering=False, debug=False,
                   enable_asserts=False, num_devices=8)
    bf_shapes = {
        "xfT": (E, N), "wqT": (E, E), "wkT": (E, 256), "wvT": (E, 256),
        "pjT": (E, E), "bpb": (128, 2, 512), "id128": (128, 128),
    }
    dr = {k: nc.dram_tensor(k, list(v), BF16, kind="ExternalInput").ap()
          for k, v in bf_shapes.items()}
    bf_shapes2 = {"ckT": (128, N), "skpT": (128, N)}
    for k, v in bf_shapes2.items():
        dr[k] = nc.dram_tensor(k, list(v), BF16, kind="ExternalInput").ap()
    for k, v in {"cqT": (128, R), "sqpT": (128, R)}.items():
        dr[k] = nc.dram_tensor(k, list(v), BF16, kind="ExternalInput").ap()
    dr["spk"] = nc.dram_tensor("spk", [128, 464], F32R,
                               kind="ExternalInput").ap()
    dr["out"] = nc.dram_tensor("out", [R, E], BF16, kind="ExternalOutput").ap()
    with tile.TileContext(nc) as tc:
        _emit(tc, dr)
    nc.compile()
    _CACHE["nc"] = nc
    return nc


def _host_prep(inputs):
    f = np.float32
    import ml_dtypes
    bf = ml_dtypes.bfloat16
    x = np.asarray(inputs["x"], f)
    sin = np.asarray(inputs["sin"], f)
    cos = np.asarray(inputs["cos"], f)
    qn_w = np.asarray(inputs["qn_w"], f)
    kn_w = np.asarray(inputs["kn_w"], f)
    d = np.arange(D)
    sw = d ^ 32
    sign = np.where(d < 32, -1.0, 1.0).astype(f)
    # cos tiles [64, N] rows indexed by d; w folded
    cq64 = (cos * qn_w).T.astype(f)
    ck64 = (cos * kn_w).T.astype(f)
    # permuted sin: sp[e, n] = -sign[e] * w[e] * sin[n, e^32]
    sq64p = (sin.T[sw, :] * (-sign * qn_w)[:, None]).astype(f)
    sk64p = (sin.T[sw, :] * (-sign * kn_w)[:, None]).astype(f)
    cq128 = np.tile(cq64, (2, 1))
    sq128p = np.tile(sq64p, (2, 1))
    ck128 = np.tile(ck64, (2, 1))
    sk128p = np.tile(sk64p, (2, 1))
    p2 = np.zeros((128, 128), f)
    i = np.arange(128)
    p2[i, (i // 64) * 64 + ((i % 64) ^ 32)] = 1.0
    bcm2 = np.zeros((2, 128), f)
    bcm2[0, 0:64] = 1.0
    bcm2[1, 64:128] = 1.0
    bcm128 = np.zeros((128, 128), f)
    for i3 in range(3):
        bcm128[32 * i3:32 * i3 + 2, :] = 8.0 * bcm2
    # 3 zero-padded column-sum masks: variant i sums into out rows 32i:32i+2
    smk = np.zeros((128, 3, 66), f)
    for i3 in range(3):
        smk[:, i3, 32 * i3:32 * i3 + 2] = bcm2.T
    # head permutation: new m index -> old m index
    perm = np.concatenate([np.arange(64 * h, 64 * h + 64) for h in HEAD_ORDER])
    wqT = np.asarray(inputs["wq_w"], f).T   # [e, m]
    pjT = np.asarray(inputs["proj_w"], f).T  # [m, mo]
    bq = np.asarray(inputs["wq_b"], f)
    # v bias folded through proj (softmax rows sum to 1), plus proj bias,
    # broadcast to all 128 partitions
    bv_full = np.asarray(inputs["wv_b"], f).reshape(KV, D)[
        np.arange(H) // (H // KV), :].reshape(E)
    bp_eff = np.asarray(inputs["proj_b"], f) + bv_full @ np.asarray(
        inputs["proj_w"], f).T
    bpb = np.tile(bp_eff.reshape(1, 2, 512), (128, 1, 1)).astype(bf)
    com = {
        "wqT": np.ascontiguousarray(wqT[:, perm]).astype(bf),
        "wkT": np.ascontiguousarray(np.asarray(inputs["wk_w"], f).T).astype(bf),
        "wvT": np.ascontiguousarray(np.asarray(inputs["wv_w"], f).T).astype(bf),
        "pjT": np.ascontiguousarray(pjT[perm, :]).astype(bf),
        "spk": np.concatenate([
            np.ascontiguousarray(
                np.asarray(inputs["wk_b"], f).reshape(2, 128).T),
            np.ascontiguousarray(bq[perm].reshape(8, 128).T),
            smk.reshape(128, 198), p2, bcm128], axis=1).astype(f),
        "bpb": bpb,
        "id128": np.eye(128, dtype=f).astype(bf),
    }
    in_maps = []
    for c in range(8):
        b, ch = c // 4, c % 4
        roff = R * ch
        m = dict(com)
        m["xfT"] = np.ascontiguousarray(np.roll(x[b].T, -roff, axis=1)).astype(bf)
        m["ckT"] = np.ascontiguousarray(np.roll(ck128, -roff, axis=1)).astype(bf)
        m["skpT"] = np.ascontiguousarray(np.roll(sk128p, -roff, axis=1)).astype(bf)
        m["cqT"] = np.ascontiguousarray(cq128[:, roff:roff + R]).astype(bf)
        m["sqpT"] = np.ascontiguousarray(sq128p[:, roff:roff + R]).astype(bf)
        in_maps.append(m)
    return in_maps


def kernel(**inputs):
    nc = _get_nc()
    in_maps = _host_prep(inputs)
    res = bass_utils.run_bass_kernel_spmd(nc, in_maps, core_ids=list(range(8)))
    out = np.empty((B, N, E), np.float32)
    for c in range(8):
        b, ch = c // 4, c % 4
        out[b, R * ch:R * (ch + 1), :] = np.asarray(
            res.results[c]["out"], np.float32)
    return out


# revision 68
# speedup vs baseline: 1.0055x; 1.0055x over previous
"""GQA attention kernel for 8 Trainium2 NeuronCores (v3).

Sharding: core c handles batch b = c//4, query rows [512*(c%4), 512*(c%4)+512).
Each core computes K/V for its batch's full (rolled) sequence, all 16 heads of
attention for its 512 query rows, and the final projection. No collectives.

v3 is a scheduling rewrite of v2 driven by TimelineSim engine occupancy:
the Act engine's exp stream (16 units x 8 x [128,1024] ~ 133us) is the hard
floor, and the PE's ~150us of matmuls must hide almost entirely under it.

  - QKV/norm/proj work is emitted as <=1us "fill granules" interleaved into
    the score units' w-slots so the PE tracks the exp stream; the first two
    head-tiles are emitted in w-halves so the Act stream backfills across
    units while the xk DMA chunks and k-norm chains land;
  - PSUM: ps 2x[128,1024] + aux 1x[128,512] (qkv/proj) + nrm 1x[128,512]
    (t2p/prb) + po 1x[128,4,65] + pkb 1x[66,512] = 16KB exactly;
  - rsqrt chains batched: up to 3 subs' sum-of-squares accumulate into one
    [66,512] PSUM tile (rows 32i, zero-padded mask stationaries) so one
    5-op DVE Newton chain serves a whole batch; prologue batches use Act
    Sqrt (before the Exp table load) instead;
  - prologue raw/sq ride the idle Act engine (Identity/Square with bias);
  - v bias folded into the output bias (softmax rows sum to 1 =>
    attn@(v+b) = attn@v + b), proj bias folded into a broadcast SBUF tile
    added during the proj_a PSUM->SBUF copy: zero bias matmuls;
  - startup DMAs chunked and ordered along the critical path; small f32
    constants packed into one [128,464] f32r transfer;
  - projection: mt0-5 accumulate as fills in units 6-7 (+ bias via bpb);
    the tail (mt6,7 + foA via identity matmul) fans out across all free
    PSUM rings, PSUM->SBUF copies alternating Act (Copy) and DVE;
  - bf16 output DMA (halves the final writeback).
"""

import numpy as np

import concourse.bass as bass
import concourse.tile as tile
from concourse import bacc, mybir
from concourse import bass_utils

B, N, E = 2, 2048, 1024
H, KV, D = 16, 4, 64
R = 512            # query rows per core
EPS = 1e-6
F32 = mybir.dt.float32
F32R = mybir.dt.float32r
U32 = mybir.dt.uint32
BF16 = mybir.dt.bfloat16
AF = mybir.ActivationFunctionType
ALU = mybir.AluOpType

# head order: tile t holds (HEAD_ORDER[2t] at rows 0:64, HEAD_ORDER[2t+1] at 64:128)
HEAD_ORDER = [0, 4, 1, 5, 2, 6, 3, 7, 8, 12, 9, 13, 10, 14, 11, 15]


def _emit(tc, dr):
    nc = tc.nc
    with (
        tc.tile_pool(name="pers", bufs=1) as pers,
        tc.tile_pool(name="work", bufs=2) as wk,
        tc.tile_pool(name="wqs", bufs=2) as wqs,
        tc.tile_pool(name="ets", bufs=21) as ets,
        tc.tile_pool(name="outs", bufs=2) as outs,
        tc.tile_pool(name="psp", bufs=2, space=bass.MemorySpace.PSUM) as psp,
        tc.tile_pool(name="auxp", bufs=1, space=bass.MemorySpace.PSUM) as auxp,
        tc.tile_pool(name="nrmp", bufs=1, space=bass.MemorySpace.PSUM) as nrmp,
        tc.tile_pool(name="pop", bufs=1, space=bass.MemorySpace.PSUM) as pop,
        tc.tile_pool(name="pkbp", bufs=1, space=bass.MemorySpace.PSUM) as pkbp,
    ):
        # ---------------- persistent tiles ----------------
        kt_til = [pers.tile([128, N], BF16, tag=f"kt{i}", name=f"ktt{i}")
                  for i in range(2)]
        qt_til = [pers.tile([128, 2, R], BF16, tag=f"qt{i}", name=f"qtt{i}")
                  for i in range(4)]
        vt_t = pers.tile([128, 16, 4, 65], BF16, tag="vt")  # v + ones col per g
        ot_t = pers.tile([128, 8, R], BF16, tag="ot")      # attn out (m, q)
        pj_t = pers.tile([128, 8, 1024], BF16, tag="pj")   # proj weights
        spk_t = pers.tile([128, 464], F32R, tag="spk")     # packed constants
        bk_t = spk_t[:, 0:2].bitcast(F32)
        bq_t = spk_t[:, 2:10].bitcast(F32)
        smk_t = spk_t[:, 10:208].rearrange("p (v c) -> p v c", v=3)
        p2_t = spk_t[:, 208:336]
        bcm_t = spk_t[:, 336:464]
        bpb_t = pers.tile([128, 2, 512], BF16, tag="bpb")  # proj+v bias bcast
        eps_t = pers.tile([128, 1], F32, tag="eps")
        kmag_t = pers.tile([128, 512], U32, tag="kmag")    # 0x5f3759df
        id_t = pers.tile([128, 128], BF16, tag="id")       # identity (tail)

        xk_t = pers.tile([128, 8, N], BF16, tag="xk")
        wk_t = pers.tile([128, 8, 256], BF16, tag="wk")
        wv_t = pers.tile([128, 8, 256], BF16, tag="wv")
        ck_t = pers.tile([128, N], BF16, tag="ck")    # cos*w for K cols
        skp_t = pers.tile([128, N], BF16, tag="skp")  # permuted sign*sin*w for K
        cq_t = pers.tile([128, R], BF16, tag="cq")
        sqp_t = pers.tile([128, R], BF16, tag="sqp")

        nc.vector.memset(vt_t[:, :, :, 64:65], 1.0)
        nc.vector.memset(kmag_t, 0x5F3759DF)
        nc.vector.memset(eps_t, 64.0 * EPS)

        # ---------- startup DMAs, ordered along the critical path ----------
        xr = dr["xfT"].rearrange("(e p) n -> p e n", p=128)
        nc.sync.dma_start(out=wk_t, in_=dr["wkT"].rearrange("(e p) m -> p e m", p=128))
        nc.sync.dma_start(out=xk_t[:, :, 0:512], in_=xr[:, :, 0:512])
        wq_c0 = wqs.tile([128, 8, 256], BF16, tag="wqc", name="wqc0")
        wqr = dr["wqT"].rearrange("(e p) m -> p e m", p=128)
        nc.sync.dma_start(out=wq_c0, in_=wqr[:, :, 0:256])
        nc.sync.dma_start(out=skp_t[:, 0:1024], in_=dr["skpT"][:, 0:1024])
        nc.sync.dma_start(out=ck_t[:, 0:1024], in_=dr["ckT"][:, 0:1024])
        nc.sync.dma_start(out=spk_t, in_=dr["spk"])
        nc.sync.dma_start(out=cq_t, in_=dr["cqT"])
        nc.sync.dma_start(out=sqp_t, in_=dr["sqpT"])
        nc.sync.dma_start(out=xk_t[:, :, 512:1024], in_=xr[:, :, 512:1024])
        nc.sync.dma_start(out=xk_t[:, :, 1024:1536], in_=xr[:, :, 1024:1536])
        nc.sync.dma_start(out=xk_t[:, :, 1536:2048], in_=xr[:, :, 1536:2048])
        nc.sync.dma_start(out=wv_t, in_=dr["wvT"].rearrange("(e p) m -> p e m", p=128))
        nc.sync.dma_start(out=ck_t[:, 1024:2048], in_=dr["ckT"][:, 1024:2048])
        nc.sync.dma_start(out=skp_t[:, 1024:2048], in_=dr["skpT"][:, 1024:2048])
        nc.sync.dma_start(out=bpb_t, in_=dr["bpb"])
        nc.sync.dma_start(out=id_t, in_=dr["id128"])

        # ---------------- norm machinery ----------------
        # Per 128-row m-block x 512-token "sub": raw = psum+bias on DVE;
        # squares (gpsimd fast / Act Square slow); sum-of-squares via a
        # zero-padded mask matmul accumulating batch row-pair 2i; rope
        # products u (gpsimd) and t1 (gpsimd); rsqrt for a whole batch in
        # one 5-op DVE Newton chain ([8,512]: free size stays 512); then
        # t2p = P@u, s = t2p+t1, prb = bcast(rsv), out = s*prb.
        class Batch:
            def __init__(self, nm, n_subs, slow):
                self.nm = nm
                self.n = n_subs
                self.slow = slow
                self.next_i = 0
                self.tile = None
                self.rsv = None

        st_u = {}
        st_t1 = {}
        st_s = {}

        def norm_front(key, pr, bias_ap, cs_ap, sp_ap, batch, act=False):
            raw = wk.tile([128, 512], F32, tag="raw", bufs=2, name=f"raw{key}")
            sq = wk.tile([128, 512], F32R, tag="sq", bufs=2, name=f"sq{key}")
            if act:
                # prologue subs: psum+bias reads on the (idle) Act engine
                nc.scalar.activation(out=raw, in_=pr, func=AF.Identity,
                                     bias=bias_ap, scale=1.0)
                nc.scalar.activation(out=sq, in_=pr, func=AF.Square,
                                     bias=bias_ap)
            else:
                nc.vector.tensor_scalar_add(out=raw, in0=pr, scalar1=bias_ap)
                nc.vector.tensor_mul(sq, raw, raw)
            i = batch.next_i
            batch.next_i += 1
            if i == 0:
                batch.tile = pkbp.tile([66, 512], F32, tag="pkb",
                                       name=f"pkb{batch.nm}")
            nc.tensor.matmul(batch.tile, smk_t[:, i, :], sq,
                             start=(i == 0), stop=(i == batch.n - 1))
            u = wk.tile([128, 512], F32R, tag="u", bufs=3, name=f"u{key}")
            nc.gpsimd.tensor_mul(u, raw, sp_ap)
            t1 = wk.tile([128, 512], F32, tag="t1", bufs=3, name=f"t1{key}")
            nc.gpsimd.tensor_mul(t1, raw, cs_ap)
            st_u[key] = u
            st_t1[key] = t1
            return i

        def chain(batch):
            pk = batch.tile
            r = 32 * (batch.n - 1) + 2
            rv = wk.tile([66, 512], F32R, tag="rv", bufs=2,
                         name=f"rv{batch.nm}")
            if batch.slow:
                sdk = wk.tile([66, 512], F32, tag="sdk", bufs=1,
                              name=f"sdk{batch.nm}")
                nc.scalar.activation(out=sdk[0:r], in_=pk[0:r], func=AF.Sqrt,
                                     bias=eps_t[0:r], scale=1.0)
                with nc.allow_low_precision(reason="bf16-level norm"):
                    nc.vector.reciprocal(out=rv[0:r], in_=sdk[0:r])
            else:
                sh = wk.tile([66, 512], U32, tag="sh", bufs=1,
                             name=f"sh{batch.nm}")
                nc.vector.tensor_scalar(out=sh[0:r], in0=pk[0:r].bitcast(U32),
                                        scalar1=1, scalar2=None,
                                        op0=ALU.logical_shift_right)
                y0 = wk.tile([66, 512], U32, tag="y0", bufs=1,
                             name=f"y0{batch.nm}")
                nc.vector.tensor_tensor(out=y0[0:r], in0=kmag_t[0:r],
                                        in1=sh[0:r], op=ALU.subtract)
                y2 = wk.tile([66, 512], F32, tag="y2", bufs=1,
                             name=f"y2{batch.nm}")
                nc.vector.tensor_mul(y2[0:r], y0[0:r].bitcast(F32),
                                     y0[0:r].bitcast(F32))
                nb = wk.tile([66, 512], F32, tag="nb", bufs=1,
                             name=f"nb{batch.nm}")
                nc.vector.scalar_tensor_tensor(out=nb[0:r], in0=pk[0:r],
                                               scalar=-0.5, in1=y2[0:r],
                                               op0=ALU.mult, op1=ALU.mult)
                nc.vector.scalar_tensor_tensor(out=rv[0:r], in0=nb[0:r],
                                               scalar=1.5,
                                               in1=y0[0:r].bitcast(F32),
                                               op0=ALU.add, op1=ALU.mult)
            batch.rsv = rv

        def norm_f2(key):
            u = st_u.pop(key)
            t2p = nrmp.tile([128, 512], F32, tag="nrm", name=f"t2p{key}")
            nc.tensor.matmul(t2p, p2_t, u, start=True, stop=True)
            s = wk.tile([128, 512], F32, tag="s", bufs=2, name=f"s{key}")
            nc.vector.scalar_tensor_tensor(out=s, in0=t2p, scalar=0.0,
                                           in1=st_t1.pop(key),
                                           op0=ALU.add, op1=ALU.add)
            st_s[key] = s

        def norm_f3(key, batch, i, out_ap):
            prb = nrmp.tile([128, 512], F32, tag="nrm", name=f"prb{key}")
            nc.tensor.matmul(prb, bcm_t[32 * i:32 * i + 2, :],
                             batch.rsv[32 * i:32 * i + 2],
                             start=True, stop=True)
            nc.vector.tensor_mul(out_ap, st_s.pop(key), prb)

        # ---------------- k / q / v sub emitters ----------------
        aux_store = {}
        sub_meta = {}

        def k_mm(kt, nb, part):
            key = f"k{kt}{nb}"
            if part == 0:
                aux_store[key] = auxp.tile([128, 512], F32, tag="aux",
                                           name=f"pk{key}")
            pr = aux_store[key]
            for e in range(4 * part, 4 * part + 4):
                nc.tensor.matmul(pr, wk_t[:, e, 128 * kt:128 * (kt + 1)],
                                 xk_t[:, e, 512 * nb:512 * (nb + 1)],
                                 start=(e == 0), stop=(e == 7))

        def k_front(kt, nb, batch, act=False):
            key = f"k{kt}{nb}"
            i = norm_front(key, aux_store.pop(key), bk_t[:, kt:kt + 1],
                           ck_t[:, 512 * nb:512 * (nb + 1)],
                           skp_t[:, 512 * nb:512 * (nb + 1)], batch, act)
            sub_meta[key] = (batch, i)

        def k_back2(kt, nb):
            norm_f2(f"k{kt}{nb}")

        def k_back3(kt, nb):
            key = f"k{kt}{nb}"
            batch, i = sub_meta.pop(key)
            norm_f3(key, batch, i,
                    kt_til[kt][:, 512 * nb:512 * (nb + 1)])

        wq_store = {0: wq_c0}

        def wq_dma(qp):
            wq_c = wqs.tile([128, 8, 256], BF16, tag="wqc", name=f"wqc{qp}")
            nc.sync.dma_start(out=wq_c, in_=wqr[:, :, 256 * qp:256 * (qp + 1)])
            wq_store[qp] = wq_c

        def q_mm(qp, j, part, pool=None):
            key = f"q{qp}{j}"
            if part == 0:
                aux_store[key] = (pool or auxp).tile(
                    [128, 512], F32, tag="po" if pool else "aux",
                    name=f"pq{key}")
            pr = aux_store[key]
            wq_c = wq_store[qp]
            for e in range(4 * part, 4 * part + 4):
                nc.tensor.matmul(pr, wq_c[:, e, 128 * j:128 * (j + 1)],
                                 xk_t[:, e, 0:R],
                                 start=(e == 0), stop=(e == 7))

        def q_front(qp, j, batch, act=False):
            key = f"q{qp}{j}"
            i = norm_front(key, aux_store.pop(key),
                           bq_t[:, 2 * qp + j:2 * qp + j + 1],
                           cq_t, sqp_t, batch, act)
            sub_meta[key] = (batch, i)

        def q_back2(qp, j):
            norm_f2(f"q{qp}{j}")

        def q_back3(qp, j):
            key = f"q{qp}{j}"
            batch, i = sub_meta.pop(key)
            norm_f3(key, batch, i, qt_til[qp][:, j, :])

        def v_mm(gp, q4, cpair):
            key = f"v{gp}{q4}"
            if cpair == 0:
                aux_store[key] = auxp.tile([128, 512], F32, tag="aux",
                                           name=f"pv{key}")
            pv = aux_store[key]
            for c in range(2 * cpair, 2 * cpair + 2):
                nch = 4 * q4 + c
                for e in range(8):
                    nc.tensor.matmul(pv[:, 128 * c:128 * (c + 1)],
                                     xk_t[:, e, 128 * nch:128 * (nch + 1)],
                                     wv_t[:, e, 128 * gp:128 * (gp + 1)],
                                     start=(e == 0), stop=(e == 7))
            if cpair == 1:
                pv = aux_store.pop(key)
                nc.vector.tensor_copy(
                    out=vt_t[:, 4 * q4:4 * q4 + 4, 2 * gp:2 * gp + 2, 0:64],
                    in_=pv.rearrange("p (c g x) -> p c g x", c=4, g=2))

        # ---------------- stage-2 unit emitters ----------------
        et_store = {}
        od_store = {}

        def score_unit(t, r01, fills=(), ws=tuple(range(8))):
            ktile = t // 4
            h = HEAD_ORDER[2 * t + r01]
            gq = h // 4
            prow = 64 * (gq % 2)
            assert gq // 2 == ktile and prow == 64 * r01
            qn_h = qt_til[t // 2][prow:prow + 64, t % 2, :]
            etl = et_store.setdefault((t, r01), [None] * 8)
            fi = 0
            for w in ws:
                ps = psp.tile([128, 1024], F32, tag="ps", name=f"ps{t}{r01}{w}")
                for c in range(2):
                    nch = 2 * w + c
                    nc.tensor.matmul(
                        ps[:, 512 * c:512 * (c + 1)],
                        kt_til[ktile][prow:prow + 64, 128 * nch:128 * (nch + 1)],
                        qn_h, start=True, stop=True)
                et = ets.tile([128, 1024], BF16, tag="et", bufs=21,
                              name=f"et{t}{r01}{w}")
                etl[w] = et
                nc.scalar.activation(out=et, in_=ps, func=AF.Exp, scale=0.125)
                if fi < len(fills):
                    fills[fi]()
                    fi += 1
            for f in fills[fi:]:
                f()

        def attnv_unit(t, r01):
            h = HEAD_ORDER[2 * t + r01]
            gq = h // 4
            etl = et_store.pop((t, r01))
            if r01 == 0:
                od_store[t] = outs.tile([128, 4, 128], BF16, tag="od",
                                        name=f"od{t}")
            od = od_store[t]
            po = pop.tile([128, 4, 65], F32, tag="po", name=f"po{t}{r01}")
            for qc in range(4):
                for nch in range(16):
                    nc.tensor.matmul(
                        po[:, qc, :],
                        etl[nch // 2][:, 512 * (nch % 2) + 128 * qc:
                                      512 * (nch % 2) + 128 * (qc + 1)],
                        vt_t[:, nch, gq, :],
                        start=(nch == 0), stop=(nch == 15))
            rcp = outs.tile([128, 4, 1], F32, tag="rcp", name=f"rcp{t}{r01}")
            for qc in range(4):
                nc.vector.reciprocal(out=rcp[:, qc, :],
                                     in_=po[:, qc, 64:65])
                nc.vector.tensor_scalar_mul(
                    out=od[:, qc, 64 * r01:64 * r01 + 64],
                    in0=po[:, qc, 0:64], scalar1=rcp[:, qc, :])

        def transp_unit(t):
            od = od_store.pop(t)
            for qc in range(4):
                nc.sync.dma_start(out=ot_t[:, t, 128 * qc:128 * (qc + 1)],
                                  in_=od[:, qc, :], transpose=True)

        # ---------------- projection ----------------
        pjr = dr["pjT"].rearrange("(m p) e -> p m e", p=128)

        def pj_dma(mp):
            nc.sync.dma_start(out=pj_t[:, 2 * mp:2 * mp + 2, :],
                              in_=pjr[:, 2 * mp:2 * mp + 2, :])

        pf_store = {}
        foA = {}

        def pa_mm(half, rc, g):
            # accumulate mt 3g..3g+2 of proj for output block (half, rc)
            if g == 0:
                pf_store[(half, rc)] = auxp.tile([128, 512], F32, tag="aux",
                                                 name=f"pf{half}{rc}")
            pf = pf_store[(half, rc)]
            for mt in range(3 * g, 3 * g + 3):
                nc.tensor.matmul(pf, ot_t[:, mt, 128 * rc:128 * (rc + 1)],
                                 pj_t[:, mt, 512 * half:512 * (half + 1)],
                                 start=(mt == 0), stop=(mt == 5))
            if g == 1:
                pf = pf_store.pop((half, rc))
                fa = outs.tile([128, 512], BF16, tag="foa", bufs=8,
                               name=f"foa{half}{rc}")
                nc.vector.tensor_tensor(out=fa, in0=pf,
                                        in1=bpb_t[:, half, :], op=ALU.add)
                foA[(half, rc)] = fa

        pt_store = {}

        def pt_start(half, rc, pool=None, ptag=None):
            pf2 = (pool or psp).tile([128, 512], F32, tag=ptag or "ps",
                                     name=f"pt{half}{rc}")
            pt_store[(half, rc)] = pf2
            nc.tensor.matmul(pf2, ot_t[:, 6, 128 * rc:128 * (rc + 1)],
                             pj_t[:, 6, 512 * half:512 * (half + 1)],
                             start=True, stop=False)

        def pt_unit(half, rc, pool=None, ptag=None, on_act=None):
            if on_act is None:
                on_act = (half == 0)
            if (half, rc) not in pt_store:
                pt_start(half, rc, pool=pool, ptag=ptag)
            pf2 = pt_store.pop((half, rc))
            nc.tensor.matmul(pf2, ot_t[:, 7, 128 * rc:128 * (rc + 1)],
                             pj_t[:, 7, 512 * half:512 * (half + 1)],
                             start=False, stop=(not on_act))
            fo = outs.tile([128, 512], BF16, tag="fo", bufs=4,
                           name=f"fo{half}{rc}")
            if on_act:
                nc.tensor.matmul(pf2, id_t, foA.pop((half, rc)),
                                 start=False, stop=True)
                nc.scalar.activation(out=fo, in_=pf2, func=AF.Copy, scale=1.0)
            else:
                nc.vector.scalar_tensor_tensor(
                    out=fo, in0=pf2, scalar=0.0, in1=foA.pop((half, rc)),
                    op0=ALU.add, op1=ALU.add)
            nc.sync.dma_start(
                out=dr["out"][128 * rc:128 * (rc + 1),
                              512 * half:512 * (half + 1)],
                in_=fo)

        # ================= schedule =================
        bS = Batch("S", 2, slow=True)      # k00, q00 (Act sqrt prologue)
        bFa = Batch("Fa", 1, slow=False)   # k01
        bFb = Batch("Fb", 2, slow=False)   # q01, k02
        bFc = Batch("Fc", 1, slow=False)   # k03
        bK1a = Batch("K1a", 2, slow=False)  # k10, k11
        bK1b = Batch("K1b", 2, slow=False)  # k12, k13
        bQ1 = Batch("Q1", 2, slow=False)   # q10, q11
        bQ2 = Batch("Q2", 2, slow=False)   # q20, q21
        bQ3 = Batch("Q3", 2, slow=False)   # q30, q31

        # ---- prologue: minimal path to the first exp ----
        # bS (k00+q00, Act sqrt) gates the first exp; k01 (chain bFa) gates
        # w=2; q01/k02/k03 are xk2/3-DMA-gated and chain later (bF).
        k_mm(0, 0, 0); k_mm(0, 0, 1)
        q_mm(0, 0, 0, pool=pop); q_mm(0, 0, 1)
        k_front(0, 0, bS, act=True); q_front(0, 0, bS, act=True)
        chain(bS)
        k_back2(0, 0); k_back3(0, 0)
        q_back2(0, 0); q_back3(0, 0)
        k_mm(0, 1, 0); k_mm(0, 1, 1); k_front(0, 1, bFa, act=True)
        chain(bFa)
        k_back2(0, 1); k_back3(0, 1)
        # first score exps for keys 0:1024 — emitted before the k02/k03
        # blocks so the Act stream backfills while xk2/3 land and chain.
        score_unit(0, 0, ws=(0, 1, 2, 3))
        q_mm(0, 1, 0); q_mm(0, 1, 1); q_front(0, 1, bFb, act=True)
        score_unit(0, 1, ws=(0, 1, 2, 3), fills=(
            lambda: v_mm(0, 0, 0),
            lambda: v_mm(0, 0, 1),
        ))
        k_mm(0, 2, 0); k_mm(0, 2, 1); k_front(0, 2, bFb, act=True)
        chain(bFb)
        q_back2(0, 1); q_back3(0, 1)
        k_back2(0, 2); k_back3(0, 2)
        score_unit(1, 0, ws=(0, 1, 2, 3), fills=(
            lambda: wq_dma(1),
            lambda: v_mm(0, 1, 0),
            lambda: v_mm(0, 1, 1),
        ))
        k_mm(0, 3, 0); k_mm(0, 3, 1); k_front(0, 3, bFc, act=True)
        chain(bFc)
        k_back2(0, 3); k_back3(0, 3)
        score_unit(1, 1, ws=(0, 1, 2, 3), fills=(
            lambda: v_mm(0, 2, 0),
            lambda: v_mm(0, 2, 1),
            lambda: v_mm(0, 3, 0),
            lambda: v_mm(0, 3, 1),
        ))
        score_unit(0, 0, ws=(4, 5, 6, 7), fills=(
            lambda: q_mm(1, 0, 0),
            lambda: (q_mm(1, 0, 1), q_front(1, 0, bQ1)),
        ))
        score_unit(0, 1, ws=(4, 5, 6, 7), fills=(
            lambda: q_mm(1, 1, 0),
            lambda: (q_mm(1, 1, 1), q_front(1, 1, bQ1)),
        ))
        attnv_unit(0, 0)
        score_unit(1, 0, ws=(4, 5, 6, 7), fills=(
            lambda: chain(bQ1),
            lambda: q_back2(1, 0),
            lambda: q_back3(1, 0),
            lambda: q_back2(1, 1),
            lambda: q_back3(1, 1),
        ))
        attnv_unit(0, 1)
        transp_unit(0)
        score_unit(1, 1, ws=(4, 5, 6, 7))
        attnv_unit(1, 0)
        score_unit(2, 0, (
            lambda: wq_dma(2),
            lambda: k_mm(1, 0, 0),
            lambda: (k_mm(1, 0, 1), k_front(1, 0, bK1a)),
            lambda: k_mm(1, 1, 0),
            lambda: (k_mm(1, 1, 1), k_front(1, 1, bK1a)),
            lambda: chain(bK1a),
            lambda: k_back2(1, 0),
            lambda: k_back3(1, 0),
        ))
        attnv_unit(1, 1)
        transp_unit(1)
        score_unit(2, 1, (
            lambda: k_back2(1, 1),
            lambda: k_back3(1, 1),
            lambda: k_mm(1, 2, 0),
            lambda: (k_mm(1, 2, 1), k_front(1, 2, bK1b)),
            lambda: k_mm(1, 3, 0),
            lambda: (k_mm(1, 3, 1), k_front(1, 3, bK1b)),
            lambda: chain(bK1b),
            lambda: k_back2(1, 2),
        ))
        attnv_unit(2, 0)
        score_unit(3, 0, (
            lambda: k_back3(1, 2),
            lambda: k_back2(1, 3),
            lambda: k_back3(1, 3),
            lambda: q_mm(2, 0, 0),
            lambda: (q_mm(2, 0, 1), q_front(2, 0, bQ2)),
            lambda: q_mm(2, 1, 0),
            lambda: (q_mm(2, 1, 1), q_front(2, 1, bQ2)),
            lambda: chain(bQ2),
        ))
        attnv_unit(2, 1)
        transp_unit(2)
        score_unit(3, 1, (
            lambda: q_back2(2, 0),
            lambda: q_back3(2, 0),
            lambda: q_back2(2, 1),
            lambda: q_back3(2, 1),
            lambda: v_mm(1, 0, 0),
            lambda: v_mm(1, 0, 1),
            lambda: v_mm(1, 1, 0),
            lambda: v_mm(1, 1, 1),
        ))
        attnv_unit(3, 0)
        score_unit(4, 0, (
            lambda: v_mm(1, 2, 0),
            lambda: v_mm(1, 2, 1),
            lambda: v_mm(1, 3, 0),
            lambda: v_mm(1, 3, 1),
            lambda: wq_dma(3),
        ))
        attnv_unit(3, 1)
        transp_unit(3)
        score_unit(4, 1)
        attnv_unit(4, 0)
        score_unit(5, 0, (
            lambda: q_mm(3, 0, 0),
            lambda: (q_mm(3, 0, 1), q_front(3, 0, bQ3)),
            lambda: q_mm(3, 1, 0),
            lambda: (q_mm(3, 1, 1), q_front(3, 1, bQ3)),
            lambda: chain(bQ3),
            lambda: q_back2(3, 0),
            lambda: q_back3(3, 0),
            lambda: q_back2(3, 1),
        ))
        attnv_unit(4, 1)
        transp_unit(4)
        score_unit(5, 1, (
            lambda: q_back3(3, 1),
            lambda: pj_dma(0),
            lambda: pj_dma(1),
        ))
        attnv_unit(5, 0)
        score_unit(6, 0, (
            lambda: pj_dma(2),
            lambda: pj_dma(3),
        ))
        attnv_unit(5, 1)
        transp_unit(5)
        score_unit(6, 1, (
            lambda: pa_mm(0, 0, 0),
            lambda: pa_mm(0, 0, 1),
            lambda: pa_mm(0, 1, 0),
            lambda: pa_mm(0, 1, 1),
            lambda: pa_mm(0, 2, 0),
            lambda: pa_mm(0, 2, 1),
            lambda: pa_mm(0, 3, 0),
            lambda: pa_mm(0, 3, 1),
        ))
        attnv_unit(6, 0)
        score_unit(7, 0, (
            lambda: pa_mm(1, 0, 0),
            lambda: pa_mm(1, 0, 1),
            lambda: pa_mm(1, 1, 0),
            lambda: pa_mm(1, 1, 1),
        ))
        attnv_unit(6, 1)
        transp_unit(6)
        score_unit(7, 1, (
            lambda: pa_mm(1, 2, 0),
            lambda: pa_mm(1, 2, 1),
            lambda: pa_mm(1, 3, 0),
            lambda: pa_mm(1, 3, 1),
        ))
        attnv_unit(7, 0)
        # pre-start tail chunks on already-free PSUM rings (mt6 accumulate)
        pt_start(0, 1, pool=nrmp, ptag="nrm")
        pt_start(0, 2, pool=auxp, ptag="aux")
        pt_start(0, 3, pool=pkbp, ptag="pkb")
        attnv_unit(7, 1)
        transp_unit(7)

        # ================= stage 3: projection tail =================
        pt_unit(0, 0)
        pt_unit(0, 1)
        pt_unit(0, 2)
        pt_unit(0, 3)
        pt_unit(1, 0, on_act=True)
        pt_unit(1, 1, pool=pop, ptag="po", on_act=True)
        pt_unit(1, 2, pool=auxp, ptag="aux", on_act=True)
        pt_unit(1, 3, pool=nrmp, ptag="nrm", on_act=True)


_CACHE = {}


def _get_nc():
    if "nc" in _CACHE:
        return _CACHE["nc"]
    nc = bacc.Bacc("TRN2", target_bir_lowering=False, debug=False,
                   enable_asserts=False, num_devices=8)
    bf_shapes = {
        "xfT": (E, N), "wqT": (E, E), "wkT": (E, 256), "wvT": (E, 256),
        "pjT": (E, E), "bpb": (128, 2, 512), "id128": (128, 128),
    }
    dr = {k: nc.dram_tensor(k, list(v), BF16, kind="ExternalInput").ap()
          for k, v in bf_shapes.items()}
    bf_shapes2 = {"ckT": (128, N), "skpT": (128, N)}
    for k, v in bf_shapes2.items():
        dr[k] = nc.dram_tensor(k, list(v), BF16, kind="ExternalInput").ap()
    for k, v in {"cqT": (128, R), "sqpT": (128, R)}.items():
        dr[k] = nc.dram_tensor(k, list(v), BF16, kind="ExternalInput").ap()
    dr["spk"] = nc.dram_tensor("spk", [128, 464], F32R,
                               kind="ExternalInput").ap()
    dr["out"] = nc.dram_tensor("out", [R, E], BF16, kind="ExternalOutput").ap()
    with tile.TileContext(nc) as tc:
        _emit(tc, dr)
    nc.compile()
    _CACHE["nc"] = nc
    return nc


def _host_prep(inputs):
    f = np.float32
    import ml_dtypes
    bf = ml_dtypes.bfloat16
    x = np.asarray(inputs["x"], f)
    sin = np.asarray(inputs["sin"], f)
    cos = np.asarray(inputs["cos"], f)
    qn_w = np.asarray(inputs["qn_w"], f)
    kn_w = np.asarray(inputs["kn_w"], f)
    d = np.arange(D)
    sw = d ^ 32
    sign = np.where(d < 32, -1.0, 1.0).astype(f)
    # cos tiles [64, N] rows indexed by d; w folded
    cq64 = (cos * qn_w).T.astype(f)
    ck64 = (cos * kn_w).T.astype(f)
    # permuted sin: sp[e, n] = -sign[e] * w[e] * sin[n, e^32]
    sq64p = (sin.T[sw, :] * (-sign * qn_w)[:, None]).astype(f)
    sk64p = (sin.T[sw, :] * (-sign * kn_w)[:, None]).astype(f)
    cq128 = np.tile(cq64, (2, 1))
    sq128p = np.tile(sq64p, (2, 1))
    ck128 = np.tile(ck64, (2, 1))
    sk128p = np.tile(sk64p, (2, 1))
    p2 = np.zeros((128, 128), f)
    i = np.arange(128)
    p2[i, (i // 64) * 64 + ((i % 64) ^ 32)] = 1.0
    bcm2 = np.zeros((2, 128), f)
    bcm2[0, 0:64] = 1.0
    bcm2[1, 64:128] = 1.0
    bcm128 = np.zeros((128, 128), f)
    for i3 in range(3):
        bcm128[32 * i3:32 * i3 + 2, :] = 8.0 * bcm2
    # 3 zero-padded column-sum masks: variant i sums into out rows 32i:32i+2
    smk = np.zeros((128, 3, 66), f)
    for i3 in range(3):
        smk[:, i3, 32 * i3:32 * i3 + 2] = bcm2.T
    # head permutation: new m index -> old m index
    perm = np.concatenate([np.arange(64 * h, 64 * h + 64) for h in HEAD_ORDER])
    wqT = np.asarray(inputs["wq_w"], f).T   # [e, m]
    pjT = np.asarray(inputs["proj_w"], f).T  # [m, mo]
    bq = np.asarray(inputs["wq_b"], f)
    # v bias folded through proj (softmax rows sum to 1), plus proj bias,
    # broadcast to all 128 partitions
    bv_full = np.asarray(inputs["wv_b"], f).reshape(KV, D)[
        np.arange(H) // (H // KV), :].reshape(E)
    bp_eff = np.asarray(inputs["proj_b"], f) + bv_full @ np.asarray(
        inputs["proj_w"], f).T
    bpb = np.tile(bp_eff.reshape(1, 2, 512), (128, 1, 1)).astype(bf)
    com = {
        "wqT": np.ascontiguousarray(wqT[:, perm]).astype(bf),
        "wkT": np.ascontiguousarray(np.asarray(inputs["wk_w"], f).T).astype(bf),
        "wvT": np.ascontiguousarray(np.asarray(inputs["wv_w"], f).T).astype(bf),
        "pjT": np.ascontiguousarray(pjT[perm, :]).astype(bf),
        "spk": np.concatenate([
            np.ascontiguousarray(
                np.asarray(inputs["wk_b"], f).reshape(2, 128).T),
            np.ascontiguousarray(bq[perm].reshape(8, 128).T),
            smk.reshape(128, 198), p2, bcm128], axis=1).astype(f),
        "bpb": bpb,
        "id128": np.eye(128, dtype=f).astype(bf),
    }
    in_maps = []
    for c in range(8):
        b, ch = c // 4, c % 4
        roff = R * ch
        m = dict(com)
        m["xfT"] = np.ascontiguousarray(np.roll(x[b].T, -roff, axis=1)).astype(bf)
        m["ckT"] = np.ascontiguousarray(np.roll(ck128, -roff, axis=1)).astype(bf)
        m["skpT"] = np.ascontiguousarray(np.roll(sk128p, -roff, axis=1)).astype(bf)
        m["cqT"] = np.ascontiguousarray(cq128[:, roff:roff + R]).astype(bf)
        m["sqpT"] = np.ascontiguousarray(sq128p[:, roff:roff + R]).astype(bf)
        in_maps.append(m)
    return in_maps


def kernel(**inputs):
    nc = _get_nc()
    in_maps = _host_prep(inputs)
    res = bass_utils.run_bass_kernel_spmd(nc, in_maps, core_ids=list(range(8)))
    out = np.empty((B, N, E), np.float32)
    for c in range(8):
        b, ch = c // 4, c % 4
        out[b, R * ch:R * (ch + 1), :] = np.asarray(
            res.results[c]["out"], np.float32)
    return out
